# revision 1
# baseline (speedup 1.0000x reference)
"""Causal scaled-dot-product attention on 8 NeuronCores (Trainium2, Bass/Tile).

Problem: x[8, 2048, 1024] f32, Wq/Wk[1024,1024], Wv[1024,512] (+biases).
  Q = xWq + bq; K = xWk + bk; V = xWv + bv
  out = softmax(causal(QK^T / sqrt(1024))) @ V          -> [8, 2048, 512] f32

Sharding: data-parallel over batch; core b handles batch element b.

Algebraic reduction (softmax is invariant to terms constant over k):
  QK^T = (xWq + bq)(xWk + bk)^T
       = x (Wq Wk^T) x^T  +  [x Wk bq]_k  +  (q-only terms, cancel in softmax)
so with M = Wq Wk^T and w = Wk bq precomputed on the host:
  softmax_k(QK^T/32) = softmax_k( (x M x^T)/32 + c ),  c[k] = x[k]·w / 32.
This replaces the separate Q and K projections with a single A = xM
projection (25% less projection compute) and kills both bias adds.

Per-core layout strategy (all matmul contractions on the partition dim):
  - host supplies xT = x[b].T as bf16 [1024, 2048], M/Wv as bf16
  - A^T[e, s] tiles: lhsT=M tile, rhs=xT;  V natural [s, o]: lhsT=xT tile
  - scores^T tile [k=128, q<=512] = sum_d xT[d,k-tile]^T @ A^T[d,q-block]
  - E^T = exp(scores^T/32 + c) (one ACT op: scale + per-partition bias +
    cast to bf16); causal handled by 0/1 mask tiles on the diagonal
    (scores are O(1) here so softmax needs no running-max subtraction)
  - Z[1, q] = ones[128,1]^T @ E^T accumulated over k-tiles (PE);
    Z^T[q,1] via a tiny matmul (lhsT=Z slice [1,128], rhs=[1,1] one);
    same trick transposes the c row vector into per-partition form
  - U[q-tile,128 x o,512] = sum_k E^T[k,q-tile]^T @ V[k-tile, o]  (PE)
  - out = U * (1/Z) + bv   (ACT per-partition scale + DVE add)
Bias bv is folded at the end (softmax rows sum to 1 exactly).

reps>1 wraps the whole compute body in a hardware For_i loop — used only by
test.py to measure per-iteration HW time free of dispatch overhead.
"""

import numpy as np
import ml_dtypes

B = 8
S = 2048
D = 1024
O = 512
P = 128
N_CORES = 8

_CACHE = {}


def _build_nc(s=S, reps=1, loop_phase="all"):
    from contextlib import ExitStack

    import concourse.tile as tile
    import concourse.mybir as mybir
    from concourse import bacc
    from concourse.bass import ds, ts

    f32 = mybir.dt.float32
    bf16 = mybir.dt.bfloat16
    AF = mybir.ActivationFunctionType

    DO = D // P            # 8 d-tiles
    EO = D // P            # 8 e-tiles
    QBLK = 512             # q-block width (moving free dim)
    NQB = s // QBLK        # q-blocks
    NKT = s // P           # k-tiles
    NSB = s // QBLK        # s-blocks in projection phase

    nc = bacc.Bacc(None, target_bir_lowering=False, debug=False)

    xT = nc.dram_tensor("xT", (D, s), bf16, kind="ExternalInput")
    m_d = nc.dram_tensor("m", (D, D), bf16, kind="ExternalInput")
    wv = nc.dram_tensor("wv", (D, O), bf16, kind="ExternalInput")
    wc_pe = nc.dram_tensor("wc_pe", (P, DO), bf16, kind="ExternalInput")
    bv_rep = nc.dram_tensor("bv_rep", (P, O), f32, kind="ExternalInput")
    mask = nc.dram_tensor("mask", (4, P, QBLK), bf16, kind="ExternalInput")
    out = nc.dram_tensor("out", (s, O), f32, kind="ExternalOutput")

    with tile.TileContext(nc) as tc, ExitStack() as ctx:
        persist = ctx.enter_context(tc.tile_pool(name="persist", bufs=1))
        wpool = ctx.enter_context(tc.tile_pool(name="wpool", bufs=1))
        etp = ctx.enter_context(tc.tile_pool(name="et", bufs=2))
        psAcc = ctx.enter_context(tc.tile_pool(name="psAcc", bufs=6, space="PSUM"))
        psZ = ctx.enter_context(tc.tile_pool(name="psZ", bufs=1, space="PSUM"))
        psT = ctx.enter_context(tc.tile_pool(name="psT", bufs=1, space="PSUM"))
        small = ctx.enter_context(tc.tile_pool(name="small", bufs=4))
        outp = ctx.enter_context(tc.tile_pool(name="outp", bufs=3))

        aT = persist.tile([P, EO, s], bf16)       # (x M)^T
        xT_sb = persist.tile([P, DO, s], bf16)    # x^T, doubles as K'^T
        v_sb = persist.tile([P, NKT, O], bf16)
        cp_sb = persist.tile([P, NKT], f32)       # c[k]/32, k-tile-major
        mask_sb = persist.tile([P, 4, QBLK], bf16)
        nc.sync.dma_start(mask_sb[:], mask.rearrange("m p q -> p m q"))
        bv_sb = persist.tile([P, O], f32)
        nc.sync.dma_start(bv_sb[:], bv_rep[:])
        wc_sb = persist.tile([P, DO], bf16)
        nc.sync.dma_start(wc_sb[:], wc_pe[:])
        ones_sb = persist.tile([P, 1], bf16)
        nc.vector.memset(ones_sb[:], 1.0)
        onef_sb = persist.tile([1, 1], f32)
        nc.vector.memset(onef_sb[:], 1.0)

        m_sb = wpool.tile([P, DO, D], bf16)
        wv_sb = wpool.tile([P, DO, O], bf16)
        m_r = m_d.rearrange("(do p) e -> p do e", p=P)
        wv_r = wv.rearrange("(do p) o -> p do o", p=P)
        xT_r = xT.rearrange("(do p) s -> p do s", p=P)
        for do in range(DO):
            nc.sync.dma_start(xT_sb[:, do], xT_r[:, do])
            nc.sync.dma_start(m_sb[:, do], m_r[:, do])
            nc.sync.dma_start(wv_sb[:, do], wv_r[:, do])

        def phase_b(_iv=None):
            # ---- Phase B: A = xM projection, V projection, c row ----
            for sb in range(NSB):
                ssl = ds(QBLK * sb, QBLK)
                for eo in range(EO):
                    ps = psAcc.tile([P, QBLK], f32, tag="acc", name="ps_a")
                    for do in range(DO):
                        nc.tensor.matmul(
                            ps[:], lhsT=m_sb[:, do, ts(eo, P)], rhs=xT_sb[:, do, ssl],
                            start=(do == 0), stop=(do == DO - 1),
                        )
                    nc.vector.tensor_copy(aT[:, eo, ssl], ps[:])
                for st in range(QBLK // P):
                    ps = psAcc.tile([P, QBLK], f32, tag="acc", name="ps_v")
                    for do in range(DO):
                        nc.tensor.matmul(
                            ps[:, :O],
                            lhsT=xT_sb[:, do, ds(QBLK * sb + P * st, P)],
                            rhs=wv_sb[:, do, :],
                            start=(do == 0), stop=(do == DO - 1),
                        )
                    nc.vector.tensor_copy(v_sb[:, sb * (QBLK // P) + st, :], ps[:, :O])
                # c row chunk: c[k] = x[k] . (Wk bq) / 32 for k in this block
                cps = psZ.tile([1, QBLK], f32, tag="zrow", name="cps")
                for do in range(DO):
                    nc.tensor.matmul(
                        cps[:], lhsT=wc_sb[:, do : do + 1], rhs=xT_sb[:, do, ssl],
                        start=(do == 0), stop=(do == DO - 1),
                    )
                c_row = small.tile([1, QBLK], f32, name="c_row")
                nc.vector.tensor_copy(c_row[:], cps[:])
                for j in range(QBLK // P):
                    kt = sb * (QBLK // P) + j
                    ctp = psT.tile([P, 1], f32, tag="tp", name="ctp")
                    nc.tensor.matmul(
                        ctp[:], lhsT=c_row[:, ts(j, P)], rhs=onef_sb[:],
                        start=True, stop=True,
                    )
                    nc.vector.tensor_copy(cp_sb[:, kt : kt + 1], ctp[:])

        def phase_c(_iv=None):
            # ---- Phase C/D: attention ----
            for qb in range(NQB):
                nkt = 4 * qb + 4
                et = etp.tile([P, NKT, QBLK], bf16, name="et")
                zps = psZ.tile([1, QBLK], f32, tag="zrow", name="zps")
                for kt in range(nkt):
                    # diagonal k-tiles only cover q >= 128*m (rest is masked out
                    # anyway); off-diagonal tiles cover the full q-block.
                    m = kt - 4 * qb
                    q0 = max(m, 0) * P
                    qw = QBLK - q0
                    qsl = ds(QBLK * qb + q0, qw)
                    ps = psAcc.tile([P, QBLK], f32, tag="acc", name="ps_s")
                    for eo in range(EO):
                        nc.tensor.matmul(
                            ps[:, :qw], lhsT=xT_sb[:, eo, ts(kt, P)], rhs=aT[:, eo, qsl],
                            start=(eo == 0), stop=(eo == EO - 1),
                        )
                    nc.scalar.activation(
                        out=et[:, kt, q0:], in_=ps[:, :qw], func=AF.Exp,
                        scale=1.0 / 32.0, bias=cp_sb[:, kt : kt + 1],
                    )
                    if m >= 0:
                        nc.vector.tensor_mul(
                            et[:, kt, q0:], et[:, kt, q0:], mask_sb[:, m, q0:]
                        )
                for kt in range(nkt):
                    q0 = max(kt - 4 * qb, 0) * P
                    nc.tensor.matmul(
                        zps[:, q0:], lhsT=ones_sb[:], rhs=et[:, kt, q0:],
                        start=(kt == 0), stop=(kt == nkt - 1), skip_group_check=True,
                    )
                z_sb = small.tile([1, QBLK], f32, name="z_sb")
                nc.vector.tensor_copy(z_sb[:], zps[:])
                for j in range(QBLK // P):
                    qs = qb * (QBLK // P) + j
                    ztp = psT.tile([P, 1], f32, tag="tp", name="ztp")
                    nc.tensor.matmul(
                        ztp[:], lhsT=z_sb[:, ts(j, P)], rhs=onef_sb[:],
                        start=True, stop=True,
                    )
                    r_sb = small.tile([P, 1], f32, name="r_sb")
                    nc.vector.reciprocal(r_sb[:], ztp[:])
                    ups = psAcc.tile([P, QBLK], f32, tag="acc", name="ups")
                    for kt in range(qs + 1):
                        nc.tensor.matmul(
                            ups[:, :O], lhsT=et[:, kt, ts(j, P)], rhs=v_sb[:, kt, :],
                            start=(kt == 0), stop=(kt == qs),
                        )
                    o_sb = outp.tile([P, O], f32, name="o_sb")
                    nc.vector.tensor_scalar_mul(o_sb[:], ups[:, :O], r_sb[:, 0:1])
                    nc.vector.tensor_add(o_sb[:], o_sb[:], bv_sb[:])
                    nc.sync.dma_start(out[ds(P * qs, P), :], o_sb[:])

        def run(phase_fns):
            if reps == 1:
                for fn in phase_fns:
                    fn()
            else:
                with tc.For_i(0, reps, 1, hint_engines=(mybir.EngineType.PE,)) as iv:
                    for fn in phase_fns:
                        fn(iv)

        if loop_phase == "all":
            run([phase_b, phase_c])
        elif loop_phase == "b":
            run([phase_b])
            phase_c()
        elif loop_phase == "c":
            phase_b()
            run([phase_c])
        else:
            raise ValueError(loop_phase)

    nc.compile()
    return nc


def _get_nc(s=S, reps=1, loop_phase="all"):
    key = (s, reps, loop_phase)
    if key not in _CACHE:
        _CACHE[key] = _build_nc(s, reps, loop_phase)
    return _CACHE[key]


def make_mask(qblk=512):
    kp = np.arange(P)[:, None]
    qf = np.arange(qblk)[None, :]
    m = np.stack([(qf >= P * i + kp) for i in range(4)], axis=0)
    return m.astype(ml_dtypes.bfloat16)


def make_in_maps(x, Wq, bq, Wk, bk, Wv, bv, s=S):
    bf = ml_dtypes.bfloat16
    x, Wq, bq, Wk, bk, Wv, bv = (
        np.asarray(a, dtype=np.float32) for a in (x, Wq, bq, Wk, bk, Wv, bv)
    )
    M = (Wq.astype(np.float64) @ Wk.T.astype(np.float64)).astype(np.float32)
    wc = ((Wk @ bq) / 32.0).astype(np.float32)
    m_b = np.ascontiguousarray(M.astype(bf))
    wv_b = np.ascontiguousarray(Wv.astype(bf))
    wc_pe = np.ascontiguousarray(wc.reshape(D // P, P).T.astype(bf))
    bv_rep = np.ascontiguousarray(np.broadcast_to(bv, (P, O)))
    mask = make_mask()
    in_maps = []
    for b in range(x.shape[0]):
        xT_b = np.ascontiguousarray(x[b].T.astype(bf))
        in_maps.append(
            dict(xT=xT_b, m=m_b, wv=wv_b, wc_pe=wc_pe, bv_rep=bv_rep, mask=mask)
        )
    return in_maps


def kernel(x, Wq, bq, Wk, bk, Wv, bv):
    from concourse.bass_utils import run_bass_kernel_spmd

    x = np.asarray(x, dtype=np.float32)
    assert x.shape == (B, S, D), x.shape
    nc = _get_nc(S)
    in_maps = make_in_maps(x, Wq, bq, Wk, bk, Wv, bv)
    res = run_bass_kernel_spmd(nc, in_maps, core_ids=list(range(N_CORES)))
    return np.stack([res.results[c]["out"] for c in range(N_CORES)], axis=0)

